# revision 30
# baseline (speedup 1.0000x reference)
"""Trainium2 Bass kernel for 16-head causal attention (B=2, S=2048, D=1024).

Returns (out, attn) matching the reference nn.Module.

Sharding: 8 cores = 2 batches x 4 head-groups (4 heads per core).
Each core computes QKV for its 4 heads, causal softmax scores in transposed
(key-major) layout, attn @ v, and a row-parallel partial of the output
projection. The attention matrix is written transposed per head; the host
transposes it back during unshard and sums the 4 output-projection partials.

Scores/probabilities are produced once, in (c, r) layout: softmax statistics
come from an appended ones-column in V (the P*V matmul's row 64 is the row
sum), normalization is a PE outer-product broadcast + DVE multiply. P tiles
are fp16 (plenty for probabilities in [0,1]); projections run in float32r.
DMA traffic alternates between the two HWDGE queues (SP / Activation).
"""

import numpy as np

import concourse.mybir as mybir
from concourse import bacc
from concourse.tile import TileContext
from concourse.bass_utils import run_bass_kernel_spmd

# Problem shapes (hardcoded per contract)
B, S, D = 2, 2048, 1024
NH, E = 16, 64          # heads, head dim
HPC = 4                 # heads per core
NCORES = 8

F32 = mybir.dt.float32
F32R = mybir.dt.float32r
F16 = mybir.dt.float16
AF = mybir.ActivationFunctionType

NEG = -10000.0          # reference mask value; exp underflows to exactly 0

RC = 512                # r-chunk width (query dim of a P-tile batch)
NRC = S // RC           # 8 r-chunks
SC = 512                # free-dim chunk for the projection matmuls
NSC = S // SC
NST = S // 128          # 16 seq tiles of 128
KD = D // 128           # 8 contraction tiles over embed dim


def build_nc():
    nc = bacc.Bacc("TRN2", target_bir_lowering=False, debug=False, num_devices=NCORES)

    # ---- per-core external inputs (host pre-sharded / pre-transposed) ----
    xT = nc.dram_tensor("xT", [D, S], F32R, kind="ExternalInput")
    wqk = nc.dram_tensor("wqk", [D, 512], F32R, kind="ExternalInput")    # [q(4x64)/8 | k(4x64)] cols
    bqk = nc.dram_tensor("bqk", [128, 4], F32, kind="ExternalInput")     # per m-tile bias columns
    wv = nc.dram_tensor("wv", [D, 4 * 65], F32R, kind="ExternalInput")   # v cols + zero col per head
    bvrow = nc.dram_tensor("bvrow", [1, 4 * 65], F32R, kind="ExternalInput")
    wo = nc.dram_tensor("wo", [HPC * E, D], F32R, kind="ExternalInput")  # (256,1024) lhsT out proj
    cmaskT = nc.dram_tensor("cmaskT", [128, 4 * RC], F16, kind="ExternalInput")  # diag masks t=0..3
    ones = nc.dram_tensor("ones", [1, 128], F32R, kind="ExternalInput")

    # ---- per-core external outputs ----
    attnT_o = nc.dram_tensor("attnT_o", [HPC, S, S], F32, kind="ExternalOutput")  # transposed per head
    outT_o = nc.dram_tensor("outT_o", [D, S], F32, kind="ExternalOutput")

    dmae = [nc.sync, nc.scalar]   # the two HWDGE queues

    with TileContext(nc) as tc:
        with (
            tc.tile_pool(name="const", bufs=1) as cpool,
            tc.tile_pool(name="wbig", bufs=1) as wpool,
            tc.tile_pool(name="qk", bufs=1) as qkpool,
            tc.tile_pool(name="vz", bufs=1) as vzpool,
            tc.tile_pool(name="pt", bufs=32) as ptpool,
            tc.tile_pool(name="ptn", bufs=2) as ptnpool,
            tc.tile_pool(name="spmp", bufs=1) as spmpool,
            tc.tile_pool(name="lr", bufs=2) as lrpool,
            tc.tile_pool(name="ob", bufs=2) as opool,
            tc.tile_pool(name="mm", bufs=1, space="PSUM") as mmps,
            tc.tile_pool(name="sps", bufs=3, space="PSUM") as sps,
            tc.tile_pool(name="zps", bufs=3, space="PSUM") as zps,
            tc.tile_pool(name="rps", bufs=1, space="PSUM") as rps,
        ):
            # ---------- load constants (split across both DMA queues) ----------
            xT_sb = cpool.tile([128, KD * S], F32R)
            wqk_sb = wpool.tile([128, KD * 512], F32R, tag="wbig")
            wv_sb = cpool.tile([128, KD * 260], F32R)
            for k in range(KD):  # per-k loads so phase 1 pipelines in as slices land
                dmae[k % 2].dma_start(out=wqk_sb[:, k * 512:(k + 1) * 512], in_=wqk[k * 128:(k + 1) * 128, :])
                dmae[(k + 1) % 2].dma_start(out=xT_sb[:, k * S:(k + 1) * S], in_=xT[k * 128:(k + 1) * 128, :])
            for k in range(KD):
                dmae[k % 2].dma_start(out=wv_sb[:, k * 260:(k + 1) * 260], in_=wv[k * 128:(k + 1) * 128, :])
            bqk_sb = cpool.tile([128, 4], F32)
            nc.sync.dma_start(out=bqk_sb[:], in_=bqk[:])
            bvrow_sb = cpool.tile([1, 260], F32R)
            nc.sync.dma_start(out=bvrow_sb[:], in_=bvrow[:])
            cmaskT_sb = cpool.tile([128, 4 * RC], F16)
            nc.scalar.dma_start(out=cmaskT_sb[:], in_=cmaskT[:])
            ones_sb = cpool.tile([1, 128], F32R)
            nc.scalar.dma_start(out=ones_sb[:], in_=ones[:])

            # ---------- phase 1: q,k projections (qkT layout: c on partition, s free) ----------
            # qk_sb m-tiles: 0 = q heads(0,1), 1 = q heads(2,3), 2 = k heads(0,1), 3 = k heads(2,3)
            qk_sb = qkpool.tile([128, 4 * S], F32R)
            for m in range(4):
                for n in range(NSC):
                    ps = mmps.tile([128, SC], F32)
                    for k in range(KD):
                        nc.tensor.matmul(
                            ps[:],
                            lhsT=wqk_sb[:, k * 512 + m * 128: k * 512 + (m + 1) * 128],
                            rhs=xT_sb[:, k * S + n * SC: k * S + n * SC + SC],
                            start=(k == 0), stop=(k == KD - 1),
                        )
                    nc.scalar.activation(
                        out=qk_sb[:, m * S + n * SC: m * S + n * SC + SC], in_=ps[:],
                        func=AF.Identity, bias=bqk_sb[:, m:m + 1],
                    )

            # ---------- phase 2: v_ext fp16 (s on partition; 65 cols per head, col 64 = ones) ----------
            v_sb = vzpool.tile([128, NST * 260], F16)
            for t in range(NST):
                ps = mmps.tile([128, 260], F32)
                for k in range(KD):
                    nc.tensor.matmul(
                        ps[:],
                        lhsT=xT_sb[:, k * S + t * 128: k * S + (t + 1) * 128],
                        rhs=wv_sb[:, k * 260:(k + 1) * 260],
                        start=(k == 0), stop=False,
                    )
                nc.tensor.matmul(  # rank-1 bias add: ones(1,128)^T @ bvrow(1,260)
                    ps[:], lhsT=ones_sb[:, 0:128], rhs=bvrow_sb[:],
                    start=False, stop=True,
                )
                nc.scalar.activation(out=v_sb[:, t * 260:(t + 1) * 260], in_=ps[:], func=AF.Copy)

            # ---------- phase 3+4 interleaved: scores/softmax/PV per r-chunk, then out-proj ----------
            # z_sb k-tiles for out-proj: tile0 rows = e of heads(0,1), tile1 = heads(2,3)
            z_sb = vzpool.tile([128, 2 * S], F32R)
            wo_sb = wpool.tile([128, 2 * D], F32R, tag="wbig")
            for k in range(2):
                dmae[k].dma_start(out=wo_sb[:, k * D:(k + 1) * D], in_=wo[k * 128:(k + 1) * 128, :])

            for j in reversed(range(NRC)):  # biggest causal block first: shortens the tail
                for hp in range(2):        # head pair
                    qoff, koff = hp * S, (2 + hp) * S
                    zp = [zps.tile([128, RC], F32, tag="zp", name=f"zp{_p}") for _p in range(2)]
                    ni = 4 * j + 4         # causal: c-tiles 0 .. 4j+3
                    pts = [[None] * ni for _ in range(2)]
                    for i in range(ni):
                        tdiag = i - 4 * j
                        for p in range(2):  # head parity; K=64 row-groups run concurrently
                            base = 64 * p
                            head = 2 * hp + p
                            sp = sps.tile([128, RC], F32)
                            nc.tensor.matmul(
                                sp[:],
                                lhsT=qk_sb[base:base + 64, koff + i * 128: koff + (i + 1) * 128],
                                rhs=qk_sb[base:base + 64, qoff + j * RC: qoff + j * RC + RC],
                                start=True, stop=True,
                            )
                            pt = ptpool.tile([128, RC], F16, tag="pt", name=f"pt{i}_{p}")
                            if tdiag >= 0:
                                spm = spmpool.tile([128, RC], F32, tag="spm")
                                nc.vector.tensor_add(
                                    out=spm[:], in0=sp[:],
                                    in1=cmaskT_sb[:, tdiag * RC:(tdiag + 1) * RC])
                                nc.scalar.activation(out=pt[:], in_=spm[:], func=AF.Exp)
                            else:
                                nc.scalar.activation(out=pt[:], in_=sp[:], func=AF.Exp)
                            pts[p][i] = pt
                            nc.tensor.matmul(
                                zp[p][0:65, :],
                                lhsT=v_sb[:, i * 260 + head * 65: i * 260 + (head + 1) * 65],
                                rhs=pt[:],
                                start=(i == 0), stop=(i == ni - 1),
                            )
                    rbs2 = []
                    for p in range(2):
                        # row 64 of zp = softmax denominator l[r]; recip lands in rbs row 0
                        rbs = lrpool.tile([128, RC], F32R, tag="rbs", name=f"rbs{p}")
                        with nc.allow_low_precision(reason="f32r rounding of softmax denom"):
                            nc.vector.reciprocal(out=rbs[0:1, :], in_=zp[p][64:65, :])
                        rb = rps.tile([128, RC], F32)
                        nc.tensor.matmul(rb[:], lhsT=ones_sb[:, 0:128], rhs=rbs[0:1, :],
                                         start=True, stop=True)
                        with nc.allow_low_precision(reason="f32r broadcast of softmax denom"):
                            nc.scalar.activation(out=rbs[:], in_=rb[:], func=AF.Copy)
                        rbs2.append(rbs)
                        # normalized z rows for this parity (ahead of attn muls: unblocks out-proj)
                        nc.vector.tensor_mul(
                            out=z_sb[64 * p:64 * p + 64, hp * S + j * RC: hp * S + j * RC + RC],
                            in0=zp[p][0:64, :], in1=rbs[0:64, :])
                    for p in range(2):
                        head = 2 * hp + p
                        rbs = rbs2[p]
                        # normalized attention, transposed layout, batched stores
                        for i0 in range(0, ni, 2):
                            g = min(2, ni - i0)
                            ptn = ptnpool.tile([128, 2 * RC], F32, tag="ptn")
                            for ii in range(g):
                                i = i0 + ii
                                eng = nc.vector if (i % 2 == 0) else nc.gpsimd
                                eng.tensor_mul(
                                    out=ptn[:, ii * RC:(ii + 1) * RC],
                                    in0=pts[p][i][:], in1=rbs[:])
                            dmae[(i0 // 2 + p) % 2].dma_start(
                                out=attnT_o[head, i0 * 128:(i0 + g) * 128, j * RC: j * RC + RC]
                                .rearrange("(g p) r -> p g r", p=128),
                                in_=ptn[:, 0:g * RC].rearrange("p (g r) -> p g r", g=g))

                if True:
                    # out-proj for the finished SC-wide column range n = j
                    n = j
                    for m in range(D // 128):
                        ps = mmps.tile([128, SC], F32)
                        for k in range(2):
                            nc.tensor.matmul(
                                ps[:],
                                lhsT=wo_sb[:, k * D + m * 128: k * D + (m + 1) * 128],
                                rhs=z_sb[:, k * S + n * SC: k * S + n * SC + SC],
                                start=(k == 0), stop=(k == 1),
                            )
                        ob = opool.tile([128, SC], F32)
                        if m % 2 == 0:
                            nc.vector.tensor_copy(out=ob[:], in_=ps[:])
                        else:
                            nc.scalar.activation(out=ob[:], in_=ps[:], func=AF.Copy)
                        dmae[m % 2].dma_start(
                            out=outT_o[m * 128:(m + 1) * 128, n * SC: n * SC + SC], in_=ob[:])

    nc.compile()
    return nc


def _host_inputs(x, W_qkv, b_qkv, W_out, b_out):
    """Build the 8 per-core input maps."""
    cc = np.arange(128, dtype=np.int64)[:, None]
    rr = np.arange(RC, dtype=np.int64)[None, :]
    cmaskT = np.empty((128, 4 * RC), dtype=np.float16)
    for t in range(4):
        cmaskT[:, t * RC:(t + 1) * RC] = np.where(rr >= cc + 128 * t, 0.0, NEG)

    in_maps = []
    for core in range(NCORES):
        b, hg = core // 4, core % 4
        h0 = hg * HPC
        qrows = W_qkv[h0 * E:(h0 + HPC) * E] / 8.0           # (256, 1024), scale folded
        krows = W_qkv[D + h0 * E: D + (h0 + HPC) * E]
        wqk_h = np.ascontiguousarray(np.concatenate([qrows, krows], 0).T)  # (1024, 512)
        bq = b_qkv[h0 * E:(h0 + HPC) * E] / 8.0
        bk = b_qkv[D + h0 * E: D + (h0 + HPC) * E]
        bqk_h = np.ascontiguousarray(np.stack([bq[:128], bq[128:], bk[:128], bk[128:]], 1))
        wv_ext = np.zeros((HPC * 65, D), dtype=np.float32)
        bv_ext = np.zeros((1, HPC * 65), dtype=np.float32)
        for lh in range(HPC):
            wv_ext[lh * 65:lh * 65 + 64] = W_qkv[2 * D + (h0 + lh) * E: 2 * D + (h0 + lh + 1) * E]
            bv_ext[0, lh * 65:lh * 65 + 64] = b_qkv[2 * D + (h0 + lh) * E: 2 * D + (h0 + lh + 1) * E]
            bv_ext[0, lh * 65 + 64] = 1.0
        wv_h = np.ascontiguousarray(wv_ext.T)                 # (1024, 260)
        wo_h = np.ascontiguousarray(W_out[:, h0 * E:(h0 + HPC) * E].T)  # (256, 1024)
        xT_h = np.ascontiguousarray(x[b].T)                   # (1024, 2048)
        in_maps.append({
            "xT": xT_h, "wqk": wqk_h, "bqk": bqk_h, "wv": wv_h, "bvrow": bv_ext,
            "wo": wo_h, "cmaskT": cmaskT,
            "ones": np.ones((1, 128), dtype=np.float32),
        })
    return in_maps


_NC_CACHE = {}


def run(x, W_qkv, b_qkv, W_out, b_out, trace=False):
    if "nc" not in _NC_CACHE:
        _NC_CACHE["nc"] = build_nc()
    nc = _NC_CACHE["nc"]
    in_maps = _host_inputs(x, W_qkv, b_qkv, W_out, b_out)
    res = run_bass_kernel_spmd(nc, in_maps, core_ids=list(range(NCORES)), trace=trace)

    out = np.empty((B, S, D), dtype=np.float32)
    attn = np.empty((B, NH, S, S), dtype=np.float32)
    for b in range(B):
        acc = None
        for hg in range(4):
            r = res.results[4 * b + hg]
            at = r["attnT_o"]                      # (HPC, S, S), transposed per head
            attn[b, hg * HPC:(hg + 1) * HPC] = np.swapaxes(at, 1, 2)
            acc = r["outT_o"] if acc is None else acc + r["outT_o"]
        out[b] = acc.T + b_out
    return (out, attn), res


def kernel(x, W_qkv, b_qkv, W_out, b_out):
    x = np.asarray(x, dtype=np.float32)
    W_qkv = np.asarray(W_qkv, dtype=np.float32)
    b_qkv = np.asarray(b_qkv, dtype=np.float32)
    W_out = np.asarray(W_out, dtype=np.float32)
    b_out = np.asarray(b_out, dtype=np.float32)
    (out, attn), _ = run(x, W_qkv, b_qkv, W_out, b_out, trace=False)
    return (out, attn)
